# revision 3
# baseline (speedup 1.0000x reference)
"""Trainium2 Bass kernel for nn_Attention_Correlation_weight_reshape_loss.

loss = [ sum|real - C_real| + sum|fake - C_fake| ] / (B*(PP^2-PP))

Key identity: C_IN == C_OUT == 0.8, so with s[b,i] = +1 if fake_weight[b,i] > 0
else -1 the fake target is rank-1:
    C_fake[b,i,j] = 0.45 + 0.35 * s[b,i] * s[b,j]
and since s*s = +/-1:
    |fake - C_fake| = | (fake - 0.45)*s_i*s_j - 0.35 |
C_real = 0.8 everywhere except the diagonal (1.0) -- the device computes
sum|x-0.8| over everything (incl. diagonal) and the host applies the tiny
diagonal correction sum(|d-1| - |d-0.8|) straight from the input array.

Per-core plan (data-parallel over batch, 8 cores x 128 batches), both maps in
the flat [batch=partition, 38416] layout -> every DMA descriptor is a 10976 B
contiguous run (near peak SDMA efficiency). Engine assignment keeps the two
streams decoupled so neither DMA queue ever stalls on the other's consumer:
  real map, 15 chunks  (ScalarE only): dispatch + in-place Abs(x - 0.8) with
      free-dim accumulate
  fake map, 15 chunks  (SyncE dispatch, VectorE compute):
      VectorE #1 (STT):    t = (x - 0.45) * s_j    via a zero-stride
                            per-partition broadcast AP of s over rows
      VectorE #2 (custom): out = |t * s_i - 0.35|, accum_out = row sums,
                            s_i broadcast along j with a zero-stride inner dim
Host sums the [128, 30] partial tensor from each core, adds the diagonal
correction, and divides by denom.
"""

from operator import add as _op_add

import numpy as np

import concourse.bacc as bacc
import concourse.bass as bass
import concourse.mybir as mybir
import concourse.tile as tile
from concourse import bass_utils
from concourse import dve_ops as _dops
from concourse.dve_spec import Spec, Src0, Src1, Zero, maxx, lower
from concourse.dve_spec import _has_src1
from concourse import dve_spec as _dspec
from concourse.dve_uop import DveOpSpec


def _ensure_axon_ntff_shim():
    """Some agent images lack antenv.axon_hooks; run_bass_kernel_spmd
    (trace=True under axon) hard-imports it. Install a minimal shim wired
    to the axon .so so tracing works instead of crashing."""
    import sys
    import types

    try:
        import antenv.axon_hooks  # noqa: F401
        return
    except ImportError:
        pass
    try:
        import antenv
    except ImportError:
        return
    mod = types.ModuleType("antenv.axon_hooks")
    _hook = [None]
    mod.set_axon_ntff_profile_hook = lambda h: _hook.__setitem__(0, h)
    mod.get_axon_ntff_profile_hook = lambda: _hook[0]
    sys.modules["antenv.axon_hooks"] = mod
    antenv.axon_hooks = mod
    try:
        from trn_agent_boot.trn_boot import _ntff_profile_via_ctypes

        mod.set_axon_ntff_profile_hook(
            _ntff_profile_via_ctypes("/opt/axon/libaxon_pjrt.so")
        )
    except Exception:
        pass


_ensure_axon_ntff_shim()

F32 = mybir.dt.float32
AF = mybir.ActivationFunctionType
ALU = mybir.AluOpType

B, PP = 1024, 196
NCORES = 8
BS = B // NCORES            # 128 batches per core
FF = PP * PP                # 38416
RC = 14                     # chunks per map
RF = FF // RC               # 2744 = 14 rows of 196
RROWS = RF // PP            # 14 rows per chunk

# chunk schedule: 13 full chunks, then a half and two quarters (finer tail
# drain: compute on earlier pieces overlaps the last transfers)
CHUNKS = [(c * RF, RF) for c in range(RC - 1)] + [
    ((RC - 1) * RF, RF // 2),
    ((RC - 1) * RF + RF // 2, RF // 2),
]
NCH = len(CHUNKS)           # 15

# output partials layout: [128, NCOL]
COL_REAL = 0                # NCH cols: per-chunk sum|x-0.8| (incl diag)
COL_FAKE = NCH              # NCH cols: per-chunk fake sums
NCOL = 2 * NCH             # 30

DENOM = float(B) * (FF - PP)

_NC_CACHE = {}


def _register_op(name, body_fn, ref_fn):
    for op in _dops.OPS:
        if op.name == name:
            return op
    spec = Spec(body=body_fn(), accum=_op_add, accum_init=Zero, reference=ref_fn)
    row = max(_dops._SUB_OPCODE_FOR_NAME.values()) + 1
    assert row < 0x20
    _dops._SUB_OPCODE_FOR_NAME[name] = row
    shas = {}
    for ver in ("v3", "v4"):
        s = DveOpSpec(
            name=name, opcode=row, uops=lower(spec, ver=ver),
            rd1_en=_has_src1(spec),
        )
        shas[ver] = s.sha(ver)
    op = _dops.DveOp(name, spec, subdim=False, uops_sha=shas)
    _dops.OPS.append(op)
    _dops.CUSTOM_DVE_SPECS[name] = spec
    return op


def _register_mul_absdiff_op():
    """out = |in0*in1 - s0|, accum_out = row-sum(out)."""

    def _body():
        e = (Src0 * Src1) - _dspec.C0
        return maxx(e, Zero - e)

    def _ref(in0, in1, c0, c1, c2):
        P = in0.shape[0]
        a = np.asarray(in0, dtype=np.float32).reshape(P, -1)
        x = np.asarray(in1, dtype=np.float32).reshape(P, -1)
        bb = np.abs(a * x - c0).astype(np.float32)
        return bb, bb.sum(axis=-1, keepdims=True)

    return _register_op("MUL_ABSDIFF_SUM_ANT", _body, _ref)


def build_nc():
    mad_op = _register_mul_absdiff_op()
    nc = bacc.Bacc(
        "TRN2", target_bir_lowering=False, debug=False, enable_asserts=False
    )
    real = nc.dram_tensor("real", [BS, FF], F32, kind="ExternalInput").ap()
    fake = nc.dram_tensor("fake", [BS, FF], F32, kind="ExternalInput").ap()
    fw = nc.dram_tensor("fw", [BS, PP], F32, kind="ExternalInput").ap()
    out = nc.dram_tensor("out", [128, NCOL], F32, kind="ExternalOutput").ap()

    with tile.TileContext(nc) as tc:
        with (
            tc.tile_pool(name="small", bufs=1) as sp,
            tc.tile_pool(name="xr", bufs=5) as xr_pool,
            tc.tile_pool(name="xfc", bufs=8) as xfc_pool,
            tc.tile_pool(name="t", bufs=2) as t_pool,
            tc.tile_pool(name="d", bufs=2) as d_pool,
        ):
            O = sp.tile([128, NCOL], F32)

            # bias constant for scalar-engine activations ([P,1] AP)
            b08 = sp.tile([128, 1], F32)
            nc.gpsimd.memset(b08[:], -0.8)

            # --- s prep: s = +/-1 from fw > 0 (small DMA via GPSIMD so the
            # HWDGE rings start streaming the big loads immediately)
            fwt = sp.tile([128, PP], F32)
            nc.gpsimd.dma_start(fwt[:], fw[:, :])
            g_t = sp.tile([128, PP], F32)
            nc.vector.tensor_scalar(g_t[:], fwt[:], 0.0, None, ALU.is_gt)
            s_t = sp.tile([128, PP], F32)
            nc.vector.tensor_scalar(s_t[:], g_t[:], 2.0, 1.0, ALU.mult, ALU.subtract)

            def real_chunk(c, lo, sz):
                xr = xr_pool.tile([128, RF], F32, tag="xr")
                # dispatch from Sync: a dispatch behind a data-dependent
                # ACTIVATE on Scalar caps the queue's lookahead at ~2 chunks
                # and starves the real DMA stream
                nc.sync.dma_start(xr[:, 0:sz], real[:, lo : lo + sz])
                nc.scalar.activation(
                    xr[:, 0:sz], xr[:, 0:sz], AF.Abs, bias=b08[:],
                    accum_out=O[:, COL_REAL + c : COL_REAL + c + 1],
                )

            def fake_chunk(c, lo, sz):
                rows = sz // PP
                r0 = lo // PP
                xfc = xfc_pool.tile([128, RF], F32, tag="xfc")
                nc.sync.dma_start(xfc[:, 0:sz], fake[:, lo : lo + sz])
                x3 = xfc[:, 0:sz].rearrange("p (i j) -> p i j", j=PP)
                # t = (x - 0.45) * s_j
                t = t_pool.tile([128, RF], F32, tag="t")
                t3 = t[:, 0:sz].rearrange("p (i j) -> p i j", j=PP)
                nc.vector.scalar_tensor_tensor(
                    t3, x3, 0.45,
                    s_t[:].rearrange("p j -> p () j").to_broadcast(
                        [128, rows, PP]
                    ),
                    ALU.subtract, ALU.mult,
                )
                # out = |t * s_i - 0.35|, accumulated along the free dims
                sib = (
                    s_t[:, r0 : r0 + rows]
                    .rearrange("p i -> p i ()")
                    .to_broadcast([128, rows, PP])
                )
                d = d_pool.tile([128, RF], F32, tag="d")
                nc.vector._custom_dve(
                    mad_op,
                    out=d[:, 0:sz].rearrange("p (i j) -> p i j", j=PP),
                    in0=t3,
                    in1=sib,
                    s0=0.35,
                    accum_out=O[:, COL_FAKE + c : COL_FAKE + c + 1],
                )

            for c, (lo, sz) in enumerate(CHUNKS):
                fake_chunk(c, lo, sz)
                real_chunk(c, lo, sz)

            nc.sync.dma_start(out[:, :], O[:])

    nc.compile()
    return nc


def _get_nc():
    if "nc" not in _NC_CACHE:
        _NC_CACHE["nc"] = build_nc()
    return _NC_CACHE["nc"]


def make_in_maps(correlation_map_real, correlation_map_fake, fake_weight):
    r = np.ascontiguousarray(correlation_map_real, dtype=np.float32).reshape(B, FF)
    f = np.ascontiguousarray(correlation_map_fake, dtype=np.float32).reshape(B, FF)
    w = np.ascontiguousarray(fake_weight, dtype=np.float32).reshape(B, PP)
    return [
        {
            "real": r[k * BS : (k + 1) * BS],
            "fake": f[k * BS : (k + 1) * BS],
            "fw": w[k * BS : (k + 1) * BS],
        }
        for k in range(NCORES)
    ], r


def diag_correction(r_flat):
    """sum(|d-1| - |d-0.8|) over the real map's diagonal entries: the device
    treats every element as target 0.8; the diagonal target is 1.0."""
    d = r_flat[:, :: PP + 1].astype(np.float64)
    return float(np.sum(np.abs(d - 1.0) - np.abs(d - 0.8)))


def reduce_outputs(results, dcorr):
    total = dcorr
    for k in range(NCORES):
        total += results[k]["out"].astype(np.float64).sum()
    return np.float32(total / DENOM)


def run(inputs, trace=False, **kwargs):
    nc = _get_nc()
    in_maps, r_flat = make_in_maps(**inputs)
    dcorr = diag_correction(r_flat)
    res = bass_utils.run_bass_kernel_spmd(
        nc, in_maps, list(range(NCORES)), trace=trace, **kwargs
    )
    return reduce_outputs(res.results, dcorr), res


def kernel(correlation_map_real, correlation_map_fake, fake_weight):
    loss, _ = run(
        dict(
            correlation_map_real=correlation_map_real,
            correlation_map_fake=correlation_map_fake,
            fake_weight=fake_weight,
        )
    )
    return loss
